# revision 5
# baseline (speedup 1.0000x reference)
"""BEiT attention block on 8 TRN2 NeuronCores, data-parallel over batch.

Full inputs -> kernel(**inputs) -> full output (16, 1025, 768) f32.

Per-core work: 2 batches of multi-head attention (N=1025 tokens, C=768,
H=12 heads, d=64) with a relative-position bias added to the logits.

Strategy (all matmul operands bf16, f32 PSUM accumulation):
  - host: transpose x -> xT (C, N), pre-transpose weights, fold the
    softmax into exp(s*scale) * exp(bias) with exp(bias^T) precomputed
    in bf16 (padded kpos rows are 0 so padded keys vanish from sums).
  - device per batch: qkvT = Wqk^T.T @ xT (q,k kept d-on-partitions),
    v in natural token-on-partitions layout with a ones column appended
    (PV then yields both attn@v and the softmax denominators).
  - scores computed transposed: sT[k, q] = k_h.T @ q_h, contraction d=64;
    the two heads of a pair sit at partitions 0-63 / 64-127 so their
    QK^T matmuls row-tile the PE array concurrently.
  - softmax denominators collected into one [24, N] tile (cross-partition
    via DMA), one batched DVE reciprocal, broadcast back via DMA with a
    partition-step-0 access pattern, applied to unnormalized outputs.
  - proj: y = a^T.T @ Wp^T + b, a^T already in the right layout.
"""

import numpy as np
import ml_dtypes

B = 16
N = 1025
C = 768
H = 12
D = 64
NCORES = 8
BPC = B // NCORES  # batches per core
NPAD = 1152        # padded key length: 9 * 128
KB = NPAD // 128   # key blocks
IB = C // 128      # input-channel blocks
QCS = [384, 384, 257]   # query chunks covering the 1025 real queries
QCO = [0, 384, 768]
SCALE = D ** -0.5
BF16 = ml_dtypes.bfloat16

_cache: dict = {}


def _build():
    import concourse.bass as bass
    import concourse.mybir as mybir
    import concourse.tile as tile
    from concourse import bacc

    dt = mybir.dt
    f32 = dt.float32
    bf = dt.bfloat16
    AFT = mybir.ActivationFunctionType

    nc = bacc.Bacc("TRN2", target_bir_lowering=False, debug=False)

    xT_d = nc.declare_dram_parameter("xT", [BPC, C, NPAD], bf, isOutput=False)
    wqk_d = nc.declare_dram_parameter("wqk", [C, 2 * C], bf, isOutput=False)
    wv_d = nc.declare_dram_parameter("wv", [C, C], bf, isOutput=False)
    wp_d = nc.declare_dram_parameter("wp", [C, C], bf, isOutput=False)
    qkb_d = nc.declare_dram_parameter("qkb", [128, 2 * IB], f32, isOutput=False)
    vb_d = nc.declare_dram_parameter("vb", [128, C], f32, isOutput=False)
    pb_d = nc.declare_dram_parameter("pb", [128, C], f32, isOutput=False)
    eb_d = nc.declare_dram_parameter("eb", [H, NPAD, N], bf, isOutput=False)
    out_d = nc.declare_dram_parameter("out", [BPC, N, C], f32, isOutput=True)

    with tile.TileContext(nc) as tc:
        from contextlib import ExitStack

        ctx = ExitStack()
        with ctx:
            consts = ctx.enter_context(tc.tile_pool(name="consts", bufs=1))
            persist = ctx.enter_context(tc.tile_pool(name="persist", bufs=1))

            # ---- constants ----
            wqk_sb = consts.tile([128, IB, 2 * C], bf)
            nc.sync.dma_start(wqk_sb[:], wqk_d.ap().rearrange("(ib p) o -> p ib o", p=128))
            wp_sb = consts.tile([128, IB, C], bf)
            nc.sync.dma_start(wp_sb[:], wp_d.ap().rearrange("(ib p) o -> p ib o", p=128))
            qkb_sb = consts.tile([128, 2 * IB], f32)
            nc.sync.dma_start(qkb_sb[:], qkb_d.ap())
            vb_sb = consts.tile([128, C], f32)
            nc.sync.dma_start(vb_sb[:], vb_d.ap())
            pb_sb = consts.tile([128, C], f32)
            nc.sync.dma_start(pb_sb[:], pb_d.ap())

            # ---- persistent per-batch tensors ----
            xT_sb = []
            v_sb = []
            a_sb = []
            for b in range(BPC):
                t = persist.tile([128, IB, NPAD], bf, name=f"xT{b}")
                nc.sync.dma_start(t[:], xT_d.ap()[b].rearrange("(ib p) q -> p ib q", p=128))
                xT_sb.append(t)
                v_sb.append(persist.tile([128, KB, H, D + 1], bf, name=f"v{b}"))
                a_sb.append(persist.tile([128, IB, NPAD], bf, name=f"a{b}"))
                # zero the padded query columns so proj sees no garbage
                nc.gpsimd.memset(a_sb[b][:, :, N:], 0.0)

            sums_sb = persist.tile([BPC * H, N], f32)

            # ---- v projection (natural layout, + ones column) ----
            with tc.tile_pool(name="vproj_ps", bufs=2, space="PSUM") as vps, \
                 tc.tile_pool(name="wv_pool", bufs=1) as wvp:
                wv_sb = wvp.tile([128, IB, C], bf)
                nc.sync.dma_start(wv_sb[:], wv_d.ap().rearrange("(ib p) o -> p ib o", p=128))
                for b in range(BPC):
                    nc.gpsimd.memset(v_sb[b][:, :, :, D:], 1.0)
                    for kpb in range(KB):
                        for vc in range(2):
                            ps = vps.tile([128, 384], f32, name="vps")
                            for ib in range(IB):
                                nc.tensor.matmul(
                                    ps[:],
                                    lhsT=xT_sb[b][:, ib, kpb * 128:(kpb + 1) * 128],
                                    rhs=wv_sb[:, ib, vc * 384:(vc + 1) * 384],
                                    start=(ib == 0),
                                    stop=(ib == IB - 1),
                                )
                            nc.vector.tensor_add(
                                out=v_sb[b][:, kpb, 6 * vc:6 * (vc + 1), :D],
                                in0=ps.rearrange("p (h d) -> p h d", d=D),
                                in1=vb_sb[:, vc * 384:(vc + 1) * 384].rearrange(
                                    "p (h d) -> p h d", d=D),
                            )

            # ---- attention over head pairs ----
            attn_ctx = ctx.enter_context(ExitStack())
            qk_pool = attn_ctx.enter_context(tc.tile_pool(name="qk", bufs=4))
            eb_pool = attn_ctx.enter_context(tc.tile_pool(name="ebp", bufs=4))
            ex_pool = attn_ctx.enter_context(tc.tile_pool(name="exp", bufs=3))
            et_pool = attn_ctx.enter_context(tc.tile_pool(name="etmp", bufs=4))
            qk_ps = attn_ctx.enter_context(tc.tile_pool(name="qk_ps", bufs=2, space="PSUM"))
            s_ps = attn_ctx.enter_context(tc.tile_pool(name="s_ps", bufs=4, space="PSUM"))
            o_ps = attn_ctx.enter_context(tc.tile_pool(name="o_ps", bufs=2, space="PSUM"))

            for hp in range(H // 2):
                # q/k projection for this head pair, both batches
                qk2 = []
                for b in range(BPC):
                    t = qk_pool.tile([128, 2, NPAD], bf, name="qk2")
                    nc.gpsimd.memset(t[:, :, N:], 0.0)
                    for sec in range(2):  # 0 = q rows, 1 = k rows
                        ocb = sec * IB + hp
                        for qc in range(3):
                            ps = qk_ps.tile([128, 384], f32, name="qkps")
                            for ib in range(IB):
                                nc.tensor.matmul(
                                    ps[:, :QCS[qc]],
                                    lhsT=wqk_sb[:, ib, sec * C + hp * 128:
                                                sec * C + (hp + 1) * 128],
                                    rhs=xT_sb[b][:, ib, QCO[qc]:QCO[qc] + QCS[qc]],
                                    start=(ib == 0),
                                    stop=(ib == IB - 1),
                                )
                            nc.scalar.activation(
                                out=t[:, sec, QCO[qc]:QCO[qc] + QCS[qc]],
                                in_=ps[:, :QCS[qc]],
                                func=AFT.Identity,
                                bias=qkb_sb[:, ocb:ocb + 1],
                            )
                    qk2.append(t)

                for qc in range(3):
                    qcs, qco = QCS[qc], QCO[qc]
                    ebt = []
                    for par in range(2):
                        h = 2 * hp + par
                        t = eb_pool.tile([128, KB, 384], bf, name="ebt")
                        nc.sync.dma_start(
                            t[:, :, :qcs],
                            eb_d.ap()[h][:, qco:qco + qcs].rearrange(
                                "(kb p) q -> p kb q", p=128),
                        )
                        ebt.append(t)
                    for b in range(BPC):
                        ex = [ex_pool.tile([128, KB, 384], bf, name="ex")
                              for _ in range(2)]
                        for kb in range(KB):
                            for par in range(2):
                                p0 = par * 64
                                st = s_ps.tile([128, 384], f32, name="st")
                                nc.tensor.matmul(
                                    st[:, :qcs],
                                    lhsT=qk2[b][p0:p0 + 64, 1, kb * 128:(kb + 1) * 128],
                                    rhs=qk2[b][p0:p0 + 64, 0, qco:qco + qcs],
                                )
                                et = et_pool.tile([128, 384], bf, name="et")
                                nc.scalar.activation(
                                    out=et[:, :qcs], in_=st[:, :qcs],
                                    func=AFT.Exp, scale=SCALE,
                                )
                                nc.vector.tensor_mul(
                                    out=ex[par][:, kb, :qcs],
                                    in0=et[:, :qcs],
                                    in1=ebt[par][:, kb, :qcs],
                                )
                        for par in range(2):
                            h = 2 * hp + par
                            po = o_ps.tile([D + 1, 384], f32, name="po")
                            for kb in range(KB):
                                nc.tensor.matmul(
                                    po[:, :qcs],
                                    lhsT=v_sb[b][:, kb, h, :],
                                    rhs=ex[par][:, kb, :qcs],
                                    start=(kb == 0),
                                    stop=(kb == KB - 1),
                                )
                            stg = et_pool.tile([65, 384], f32, name="stg")
                            nc.vector.tensor_copy(
                                out=stg[64:65, :qcs], in_=po[D:D + 1, :qcs])
                            nc.sync.dma_start(
                                sums_sb[b * H + h:b * H + h + 1, qco:qco + qcs],
                                stg[64:65, :qcs],
                            )
                            nc.scalar.activation(
                                out=a_sb[b][par * 64:(par + 1) * 64, hp,
                                            qco:qco + qcs],
                                in_=po[:D, :qcs],
                                func=AFT.Copy,
                            )

            attn_ctx.close()

            # ---- softmax denominators: batched reciprocal + broadcast ----
            recip_sb = persist.tile([BPC * H, N], f32)
            nc.vector.reciprocal(recip_sb[:], sums_sb[:])
            recip_bf = persist.tile([BPC * H, N], bf)
            nc.vector.tensor_copy(out=recip_bf[:], in_=recip_sb[:])
            recip_dram = nc.dram_tensor("recip_dram", [BPC * H, N], bf)
            nc.sync.dma_start(recip_dram.ap(), recip_bf[:])

            rb_pool = ctx.enter_context(tc.tile_pool(name="rb", bufs=2))
            for b in range(BPC):
                for h in range(H):
                    p0 = (h % 2) * 64
                    rb = rb_pool.tile([128, N], bf, name="rb")
                    src = recip_dram.ap()[b * H + h:b * H + h + 1, :]
                    bcast = bass.AP(
                        tensor=src.tensor,
                        offset=src.offset,
                        ap=[[0, 64]] + list(src.ap[1:]),
                    )
                    nc.sync.dma_start(rb[p0:p0 + 64, :], bcast)
                    nc.vector.tensor_mul(
                        out=a_sb[b][p0:p0 + 64, h // 2, :N],
                        in0=a_sb[b][p0:p0 + 64, h // 2, :N],
                        in1=rb[p0:p0 + 64, :],
                    )

            # ---- output projection ----
            with tc.tile_pool(name="y_ps", bufs=2, space="PSUM") as yps, \
                 tc.tile_pool(name="y_sb", bufs=3) as ysb:
                for b in range(BPC):
                    for qb in range(KB):
                        rows = min(128, N - qb * 128)
                        yt = ysb.tile([128, C], f32, name="yt")
                        for oc2 in range(2):
                            ps = yps.tile([128, 384], f32, name="yps")
                            for ib in range(IB):
                                nc.tensor.matmul(
                                    ps[:],
                                    lhsT=a_sb[b][:, ib, qb * 128:(qb + 1) * 128],
                                    rhs=wp_sb[:, ib, oc2 * 384:(oc2 + 1) * 384],
                                    start=(ib == 0),
                                    stop=(ib == IB - 1),
                                )
                            nc.vector.tensor_add(
                                out=yt[:, oc2 * 384:(oc2 + 1) * 384],
                                in0=ps[:],
                                in1=pb_sb[:, oc2 * 384:(oc2 + 1) * 384],
                            )
                        nc.sync.dma_start(
                            out_d.ap()[b][qb * 128:qb * 128 + rows, :],
                            yt[:rows, :],
                        )

    nc.compile()
    return nc


def _prepare_inputs(x, qkv_weight, q_bias, v_bias, rel_pos_table, proj_weight,
                    proj_bias, rel_pos_index):
    x = np.asarray(x, np.float32)
    qkv_weight = np.asarray(qkv_weight, np.float32)
    q_bias = np.asarray(q_bias, np.float32)
    v_bias = np.asarray(v_bias, np.float32)
    rel_pos_table = np.asarray(rel_pos_table, np.float32)
    proj_weight = np.asarray(proj_weight, np.float32)
    proj_bias = np.asarray(proj_bias, np.float32)
    rel_pos_index = np.asarray(rel_pos_index)

    wqk = np.ascontiguousarray(qkv_weight[:2 * C].T).astype(BF16)
    wv = np.ascontiguousarray(qkv_weight[2 * C:].T).astype(BF16)
    wp = np.ascontiguousarray(proj_weight.T).astype(BF16)

    qkb = np.concatenate([q_bias, np.zeros(C, np.float32)])
    qkb = np.ascontiguousarray(qkb.reshape(2 * IB, 128).T)  # [128, 12]
    vb = np.ascontiguousarray(np.broadcast_to(v_bias, (128, C)))
    pb = np.ascontiguousarray(np.broadcast_to(proj_bias, (128, C)))

    # exp of transposed rel-pos bias, padded key rows = 0
    bias_qkh = rel_pos_table[rel_pos_index.reshape(-1)].reshape(N, N, H)
    ebt = np.zeros((H, NPAD, N), BF16)
    ebt[:, :N, :] = np.exp(bias_qkh.transpose(2, 1, 0).astype(np.float64)).astype(BF16)

    in_maps = []
    for core in range(NCORES):
        xb = x[core * BPC:(core + 1) * BPC]
        xT = np.zeros((BPC, C, NPAD), BF16)
        xT[:, :, :N] = xb.transpose(0, 2, 1).astype(BF16)
        in_maps.append({
            "xT": xT, "wqk": wqk, "wv": wv, "wp": wp,
            "qkb": qkb, "vb": vb, "pb": pb, "eb": ebt,
        })
    return in_maps


def kernel(**inputs) -> np.ndarray:
    from concourse.bass_utils import run_bass_kernel_spmd

    if "nc" not in _cache:
        _cache["nc"] = _build()
    nc = _cache["nc"]

    in_maps = _prepare_inputs(**inputs)
    trace = bool(_cache.get("trace", False))
    res = run_bass_kernel_spmd(nc, in_maps, core_ids=list(range(NCORES)),
                               trace=trace)
    _cache["last_results"] = res
    out = np.concatenate([r["out"] for r in res.results], axis=0)
    return out.astype(np.float32)
